# revision 2
# baseline (speedup 1.0000x reference)
"""Euclidean contrastive loss on 8 Trainium2 NeuronCores (Bass/Tile) — v2.

Triangle-sharded SPMD strategy (one program, per-core rotated inputs):
  - Host: tokens cast bf16 and TRANSPOSED to [D, N]; per core c rotate cols by
    c*1024 and keep the first 5120 local cols (10 chunks of 512). Labels become
    host-built one-hots (OHT by class x col, OHB by row x class per block).
  - The full [N, N] distance matrix is symmetric: each core computes 18 of its
    32 [512, 512] cells — for each of its two 512-row chunks r, the col chunks
    at offsets d=0..8. Cells d=1..7 are counted twice (row-sums via ACT exp
    accumulator + col-sums via PE ones-matmul on the exp tiles, masked-dist
    sums doubled on host); the diagonal (d=0) and antipodal (d=8) cells once.
  - Device per core:
      * normalize in transposed layout: DVE square -> PE ones-matmul col sums
        -> GPSIMD partition_broadcast -> ACT sqrt -> DVE reciprocal -> DVE
        scale of the 4 tT k-tiles. No HBM bounce, no DMA transpose.
      * sim = tT.T @ tT per (128-row block x 1536-col group) in PSUM fp32;
        diag fix sim[ii] -= 2; dist/tau = ACT Sqrt(A - A*sim) -> fp16.
      * masked dist sums via PE one-hot matmul + fused DVE (T * OHT) row-accum.
      * E = exp(-dist) into bf16 tiles (fp16 would underflow at e^-20) with
        fp32 row-sum accumulation; col-sums via PE ones-matmul -> DVE copy ->
        DMA out.
  - Host: scatter-add row/col partial sums, LSE = log(rowsum), npos from
    labels; loss = (masked + sum npos*LSE - N*2/tau) / sum(npos).
"""

import os
import sys

import numpy as np
import ml_dtypes

try:
    import concourse.bass as bass  # noqa: F401
except ImportError:  # harness runs from a bare directory
    for p in ("/opt/trn_rl_repo", os.path.expanduser("~/.axon_site/_ro/trn_rl_repo")):
        if os.path.isdir(p) and p not in sys.path:
            sys.path.insert(0, p)
    import concourse.bass as bass  # noqa: F401

import concourse.mybir as mybir
import concourse.tile as tile
from concourse import bacc, bass_utils
from concourse.tile import add_dep_helper

N, D, NCORES = 8192, 512, 8
RPC = N // NCORES        # 1024 rows per core
NB = RPC // 128          # 8 row blocks of 128
KT = D // 128            # 4 contraction tiles
CH = 512                 # col chunk
NCHL = 10                # local col chunks used (0..9)
SPAN = NCHL * CH         # 5120 local cols
GW = 1536                # sim col group width (3 psum banks)
NCLS = 100               # label classes

# cells: (row-chunk r in {0,1}, local col chunk ch). weight 1 for diagonal
# (ch==r) and antipodal (ch==r+8) cells, else 2 (mirror counted via col-sums).
CELLS = [(0, ch) for ch in range(9)] + [(1, ch) for ch in range(1, 10)]
CELL_W = [1.0 if (ch == r or ch == r + 8) else 2.0 for (r, ch) in CELLS]
CS_CELLS = [(0, ch) for ch in range(1, 8)] + [(1, ch) for ch in range(2, 9)]
# T-matmul chunk sets (3 chunks per [128,1536] psum tile)
T_SETS = {0: [[0, 1, 2], [3, 4, 5], [6, 7, 8]], 1: [[1, 2, 3], [4, 5, 6], [7, 8, 9]]}
# colsum chunk sets (2 chunks per [1,1024] psum tile)
C_SETS = {0: [[1, 2], [3, 4], [5, 6], [7]], 1: [[2, 3], [4, 5], [6, 7], [8]]}

BF16 = mybir.dt.bfloat16
FP8 = mybir.dt.float8e4
FP16 = mybir.dt.float16
FP32 = mybir.dt.float32
OP = mybir.AluOpType
AF = mybir.ActivationFunctionType

_CACHE: dict = {}
last_results = None  # test harness reads exec_time_ns from here


def _build(tau: float):
    nc = bacc.Bacc(
        "TRN2",
        target_bir_lowering=False,
        debug=False,
        enable_asserts=False,
        num_devices=NCORES,
    )
    tok8d = nc.dram_tensor("tok8", [D, SPAN], FP8, kind="ExternalInput")
    ohtd = nc.dram_tensor("oht", [128, SPAN], BF16, kind="ExternalInput")
    ohbd = nc.dram_tensor("ohb", [128, NB * NCLS], FP16, kind="ExternalInput")
    outd = nc.dram_tensor("part", [128, 8 + len(CELLS)], FP32, kind="ExternalOutput")
    colpd = nc.dram_tensor("colp", [1, len(CS_CELLS) * CH], FP32, kind="ExternalOutput")

    A = 2.0 / (tau * tau)  # (dist/tau)^2 = A - A*sim

    act_chain = []  # ACT instructions in required execution order

    def act(*args, **kwargs):
        inst = nc.scalar.activation(*args, **kwargs)
        act_chain.append(inst)
        return inst

    with tile.TileContext(nc) as tc:
        with (
            tc.tile_pool(name="persist", bufs=1) as pp,
            tc.tile_pool(name="dist", bufs=8) as distp,
            tc.tile_pool(name="ep", bufs=4) as ep,
            tc.tile_pool(name="scratch", bufs=2) as sc,
            tc.tile_pool(name="psum", bufs=2, space="PSUM") as psum,
            tc.tile_pool(name="ccb", bufs=2) as ccp,
            tc.tile_pool(name="wpsum", bufs=1, space="PSUM") as wpsum,
            tc.tile_pool(name="init", bufs=1) as initp,
        ):
            # ---- persistent tiles ----
            h8 = [
                pp.tile([128, SPAN], FP8, tag=f"h8_{k}", name=f"h8_{k}")
                for k in range(KT)
            ]
            OHT = pp.tile([128, SPAN], BF16, tag="OHT")
            OHB = pp.tile([128, NB * NCLS], FP16, tag="OHB")
            nrmv = pp.tile([128, SPAN], FP16, tag="nrmv")
            tT8 = [
                pp.tile([128, 2, SPAN], FP8, tag=f"tT8_{t}", name=f"tT8_{t}")
                for t in range(2)
            ]
            dms = pp.tile([128, 4 * 512], BF16, tag="dms")
            idK = pp.tile([128, 128], BF16, tag="idK")
            outp = pp.tile([128, 8 + len(CELLS)], FP32, tag="outp")
            biasA = pp.tile([128, 1], FP32, tag="biasA")
            biasL = pp.tile([128, 1], FP32, tag="biasL")
            ones128 = pp.tile([128, 128], FP16, tag="ones128")
            onesbf = pp.tile([128, 128], BF16, tag="onesbf")

            nc.gpsimd.memset(biasA[:], float(A))
            nc.gpsimd.memset(biasL[:], 2.772588722239781)
            nc.gpsimd.memset(ones128[:], 1.0)
            nc.gpsimd.memset(onesbf[:], 1.0)

            # ---- diag masks dm_k[p, f] = (f - p == 128k) ----
            iot = initp.tile([128, 512], mybir.dt.int32, tag="iot")
            nc.gpsimd.iota(iot[:], pattern=[[1, 512]], base=0, channel_multiplier=-1)
            iotf = initp.tile([128, 512], FP32, tag="iotf")
            nc.vector.tensor_copy(iotf[:], iot[:])
            for kk in range(4):
                nc.vector.tensor_scalar(
                    dms[:, kk * 512:(kk + 1) * 512], iotf[:],
                    float(kk * 128), None, op0=OP.is_equal,
                )
            nc.vector.tensor_scalar_mul(dms[:], dms[:], -512.0)
            nc.vector.tensor_scalar(
                idK[:], iotf[:, 0:128], 0.0, None, op0=OP.is_equal,
            )

            # ---- load tT + norms (GPSIMD square -> PE col sums, broadcast
            # across partitions via ones[128,128] lhsT; inv = exp(-ln(n2)/2)) ----
            for h in range(2):
                hs = slice(h * 2560, (h + 1) * 2560)
                for k in range(KT):
                    nc.sync.dma_start(
                        h8[k][:, hs], tok8d[k * 128:(k + 1) * 128, hs],
                    )
            nc.sync.dma_start(OHT[:], ohtd[:, :])
            nc.sync.dma_start(OHB[:], ohbd[:, :])
            sqs = []
            for k in range(KT):
                sq = distp.tile([128, SPAN], FP16, tag="dist", name=f"sq{k}")
                sqs.append(sq)
            for h in range(2):
                hs = slice(h * 2560, (h + 1) * 2560)
                for k in range(KT):
                    if k % 2 == 0:
                        act(sqs[k][:, hs], h8[k][:, hs], AF.Square)
                    else:
                        nc.vector.tensor_mul(sqs[k][:, hs], h8[k][:, hs],
                                             h8[k][:, hs])
            for gi in range(4):  # 10 chunks in groups of 3 (last has 1)
                chs = [c for c in range(gi * 3, min(gi * 3 + 3, NCHL))]
                w = len(chs) * 512
                ps = psum.tile([128, GW], FP32, tag="ps", name=f"nps{gi}")
                for ci, c in enumerate(chs):
                    for k in range(KT):
                        nc.tensor.matmul(
                            ps[:, ci * 512:(ci + 1) * 512],
                            ones128[:, :],
                            sqs[k][:, c * 512:(c + 1) * 512],
                            start=(k == 0), stop=(k == KT - 1),
                        )
                act(nrmv[:, gi * GW:gi * GW + w], ps[:, 0:w], AF.Ln)
            for h in range(2):  # inv = exp(-0.5*ln(n2)), split for overlap
                hs = slice(h * 2560, (h + 1) * 2560)
                act(nrmv[:, hs], nrmv[:, hs], AF.Exp, scale=-0.5,
                    bias=biasL[:])
            for si, (h0, h1) in enumerate(((0, 1536), (1536, 3456), (3456, 5120))):
                hs = slice(h0, h1)
                for k in range(KT):
                    nc.vector.scalar_tensor_tensor(
                        out=tT8[k // 2][:, k % 2, hs],
                        in0=h8[k][:, hs], scalar=1.0,
                        in1=nrmv[:, hs], op0=OP.mult, op1=OP.mult,
                    )

            # ---- main compute ----
            dist_of = {}
            E_of = {}

            def sim_round(r, groups=(0, 1, 2)):
                off = 512 * r
                if 0 in groups:
                    for m in range(4 * r, 4 * r + 4):
                        dist_of[m] = distp.tile([128, SPAN], FP16, tag="dist",
                                                name=f"dist{m}")
                for g in groups:
                    goff = off + g * GW
                    for m in range(4 * r, 4 * r + 4):
                        dist_m = dist_of[m]
                        ps = psum.tile([128, GW], FP32, tag="ps",
                                       name=f"ps{m}_{g}")
                        for t in range(2):
                            lhsT = tT8[t][:, :, m * 128:(m + 1) * 128]
                            for s in range(GW // 512):
                                nc.tensor.matmul(
                                    ps[:, s * 512:(s + 1) * 512],
                                    lhsT,
                                    tT8[t][:, :, goff + s * 512: goff + (s + 1) * 512],
                                    start=(t == 0),
                                    stop=(t == 1) and not (g == 0 and s == 0),
                                    perf_mode=mybir.MatmulPerfMode.DoubleRow,
                                )
                        if g == 0:
                            # diag fix via PE: accumulate idK.T @ (-512*mask)
                            nc.tensor.matmul(
                                ps[:, 0:512], idK[:, :],
                                dms[:, (m % 4) * 512:(m % 4 + 1) * 512],
                                start=False, stop=True,
                            )
                        act(dist_m[:, goff:goff + GW], ps[:, :], AF.Sqrt,
                            bias=biasA[:], scale=float(-A / 256.0))

            def t_round(r):
                blocks = list(range(4 * r, 4 * r + 4))
                for si, tset in enumerate(T_SETS[r]):
                    tps = psum.tile([128, GW], FP32, tag="ps",
                                    name=f"tps{r}_{si}")
                    for mi, m in enumerate(blocks):
                        for ci, c in enumerate(tset):
                            nc.tensor.matmul(
                                tps[0:NCLS, ci * 512:(ci + 1) * 512],
                                OHB[:, m * NCLS:(m + 1) * NCLS],
                                dist_of[m][:, c * 512:(c + 1) * 512],
                                start=(mi == 0), stop=(mi == 3),
                            )
                    for ci, c in enumerate(tset):
                        cell = CELLS.index((r, c))
                        tjunk = sc.tile([128, 512], FP16, tag="tjunk")
                        nc.vector.scalar_tensor_tensor(
                            out=tjunk[0:NCLS, :],
                            in0=tps[0:NCLS, ci * 512:(ci + 1) * 512],
                            scalar=1.0,
                            in1=OHT[0:NCLS, c * 512:(c + 1) * 512],
                            op0=OP.mult, op1=OP.mult,
                            accum_out=outp[0:NCLS, 8 + cell:9 + cell],
                        )

            def exp_block(m):
                off = 512 * (m // 4)
                E_m = ep.tile([128, SPAN], BF16, tag="E", name=f"E{m}")
                E_of[m] = E_m
                e = act(E_m[:, off:off + 9 * 512],
                        dist_of[m][:, off:off + 9 * 512], AF.Exp, scale=-1.0,
                        accum_out=outp[:, m:m + 1])
                # HAM keep-warm: tiny matmul pinned after this exp so the PE
                # clock gate stays open through the ACT-bound window
                wps = wpsum.tile([128, 512], FP32, tag="wps", name=f"wps{m}")
                w = nc.tensor.matmul(wps[:, :], idK[:, :], dms[:, 0:512],
                                     start=True, stop=True)
                add_dep_helper(w.ins, e.ins, reason="ham keep-warm")

            def exp_round(r):
                for m in range(4 * r, 4 * r + 4):
                    exp_block(m)

            def colsum_round(r):
                blocks = list(range(4 * r, 4 * r + 4))
                chunks = [c for cset in C_SETS[r] for c in cset]  # 7 contiguous
                for si in range(3):  # 3+3+1 chunk slices per psum tile
                    cset = chunks[si * 3:si * 3 + 3]
                    if not cset:
                        continue
                    w = len(cset) * 512
                    cps = psum.tile([128, GW], FP32, tag="ps",
                                    name=f"cps{r}_{si}")
                    for ci, c in enumerate(cset):
                        for mi, m in enumerate(blocks):
                            nc.tensor.matmul(
                                cps[:, ci * 512:(ci + 1) * 512],
                                onesbf[:, :],
                                E_of[m][:, c * 512:(c + 1) * 512],
                                start=(mi == 0), stop=(mi == 3),
                            )
                    cc = ccp.tile([1, GW], FP32, tag="cc")
                    nc.vector.tensor_copy(cc[0:1, 0:w], cps[0:1, 0:w])
                    slot = CS_CELLS.index((r, cset[0]))
                    nc.sync.dma_start(
                        colpd[0:1, slot * 512:slot * 512 + w], cc[0:1, 0:w],
                    )

            sim_round(0)
            t_round(0)
            sim_round(1, groups=(0,))
            exp_round(0)
            sim_round(1, groups=(1, 2))
            colsum_round(0)
            t_round(1)
            exp_round(1)
            colsum_round(1)

            nc.sync.dma_start(outd[:, :], outp[:])

            # ---- pin ACT execution order (stop table-set thrash) ----
            for a, b in zip(act_chain, act_chain[1:]):
                add_dep_helper(b.ins, a.ins, reason="act table-set order")

    nc.compile()
    return nc


def _get_program(tau: float):
    if tau not in _CACHE:
        _CACHE[tau] = _build(tau)
    return _CACHE[tau]


def make_in_maps(tokens: np.ndarray, labels: np.ndarray):
    bf = ml_dtypes.bfloat16
    f8 = ml_dtypes.float8_e4m3fn
    tokT_full = (np.ascontiguousarray(
        np.asarray(tokens, dtype=np.float32).T
    ) * 16.0).astype(f8)                           # [D, N] fp8, x16
    lab = np.asarray(labels).astype(np.int64)
    vids = np.arange(128)
    in_maps = []
    for c in range(NCORES):
        sh = c * RPC
        t8 = np.ascontiguousarray(np.roll(tokT_full, -sh, axis=1)[:, :SPAN])
        lab_loc = np.roll(lab, -sh)[:SPAN]
        oht = (vids[:, None] == lab_loc[None, :]).astype(bf)
        lab_blk = lab_loc[:RPC].reshape(NB, 128)
        ohb = np.zeros((128, NB * NCLS), dtype=np.float16)
        for m in range(NB):
            ohb[:, m * NCLS:(m + 1) * NCLS] = (
                lab_blk[m][:, None] == np.arange(NCLS)[None, :]
            )
        in_maps.append({"tok8": t8, "oht": oht, "ohb": ohb})
    return in_maps


def _install_ntff_hook_shim():
    """Provide antenv.axon_hooks if the image lacks it (NTFF profiling via
    direct ctypes calls into libaxon_pjrt.so)."""
    try:
        from antenv.axon_hooks import get_axon_ntff_profile_hook  # noqa: F401
        return True
    except ImportError:
        pass
    so_path = "/opt/axon/libaxon_pjrt.so"
    if not os.path.exists(so_path):
        return False
    import contextlib
    import ctypes
    import types

    lib = ctypes.CDLL(so_path)
    if not hasattr(lib, "axon_start_nrt_profile"):
        return False
    lib.axon_start_nrt_profile.argtypes = [
        ctypes.POINTER(ctypes.c_int64), ctypes.c_size_t,
    ]
    lib.axon_start_nrt_profile.restype = ctypes.c_int64
    lib.axon_stop_nrt_profile.argtypes = [ctypes.c_char_p]
    lib.axon_stop_nrt_profile.restype = ctypes.c_int64

    @contextlib.contextmanager
    def _hook(output_dir, device_ids):
        import jax
        jax.devices()
        if device_ids:
            ids = (ctypes.c_int64 * len(device_ids))(*device_ids)
            rc = lib.axon_start_nrt_profile(ids, len(device_ids))
        else:
            rc = lib.axon_start_nrt_profile(None, 0)
        if rc != 0:
            raise RuntimeError(f"axon_start_nrt_profile rc={rc}")
        try:
            yield
        finally:
            n = lib.axon_stop_nrt_profile(str(output_dir).encode())
            if n < 0:
                raise RuntimeError(f"axon_stop_nrt_profile rc={n}")
            print(f"profile: {n} file(s) written to {output_dir}")

    mod = types.ModuleType("antenv.axon_hooks")
    mod.get_axon_ntff_profile_hook = lambda: _hook
    mod.set_axon_ntff_profile_hook = lambda h: None
    sys.modules["antenv.axon_hooks"] = mod
    return True


def kernel(tokens, labels, temperature=0.07):
    global last_results
    tau = float(temperature)
    nc = _get_program(tau)
    lab = np.asarray(labels).astype(np.int64)
    in_maps = make_in_maps(tokens, lab)
    trace = bool(int(os.environ.get("KBENCH_TRACE", "0")))
    if trace:
        trace = _install_ntff_hook_shim()
    res = bass_utils.run_bass_kernel_spmd(
        nc, in_maps, core_ids=list(range(NCORES)),
        trace=trace,
    )
    last_results = res

    rs_g = np.zeros(N, dtype=np.float64)
    num = 0.0
    for c in range(NCORES):
        sh = c * RPC
        part = res.results[c]["part"].astype(np.float64)
        colp = res.results[c]["colp"].astype(np.float64)
        rs_g[sh:sh + RPC] += part[:, 0:8].T.reshape(RPC)
        for slot, (r, ch) in enumerate(CS_CELLS):
            gcols = (sh + ch * 512 + np.arange(512)) % N
            rs_g[gcols] += colp[0, slot * 512:(slot + 1) * 512]
        for ci, (r, ch) in enumerate(CELLS):
            num += CELL_W[ci] * part[0:NCLS, 8 + ci].sum()
    cnt = np.bincount(lab, minlength=NCLS)
    npos = (cnt[lab] - 1).astype(np.float64)
    num += (npos * np.log(rs_g)).sum() - N * (2.0 / tau)
    den = npos.sum()
    return np.float32(num / den)


# revision 3
# speedup vs baseline: 1.2723x; 1.2723x over previous
"""Euclidean contrastive loss on 8 Trainium2 NeuronCores (Bass/Tile) — v2.

Triangle-sharded SPMD strategy (one program, per-core rotated inputs):
  - Host: tokens transposed to [D, N], scaled x16, cast fp8e4m3; per core c
    rotate cols by c*1024 and keep the first 5120 local cols (10 chunks of
    512). Labels become host-built one-hots (OHT class x col, OHB row x class).
  - The full [N, N] distance matrix is symmetric: each core computes 18 of its
    32 [512, 512] cells — for each of its two 512-row chunks r, the col chunks
    at offsets d=0..8. Cells d=1..7 are counted twice (row-sums via ACT exp
    accumulator + col-sums via PE ones-matmul on the exp tiles, masked-dist
    sums doubled on host); the diagonal (d=0) and antipodal (d=8) cells once.
  - Device per core:
      * norms in transposed layout: squares (ACT/DVE), column sums via PE
        ones[128,128]-matmul (result broadcast across partitions in PSUM),
        inv = exp(-0.5*ln(n2) + ln 16) on ACT; normalized fp8 tiles written by
        DVE stt in column slices, interleaved k-pairs for DoubleRow.
      * sim = tT.T @ tT in fp8 DoubleRow (2 k-pair matmuls per [128,1536]
        PSUM group, 256x scaling); diagonal fix injected as a third
        accumulating matmul (identity lhsT x (-512*mask)) so no DVE op sits
        between PE and the ACT sqrt; dist/tau = Sqrt(A - A*sim/256) -> fp16.
      * masked dist sums via PE one-hot matmul + fused DVE (T * OHT) row-accum.
      * E = exp(-dist) into bf16 tiles (fp16 would underflow at e^-20) with
        fp32 row-sum accumulation; col-sums via PE ones-matmul -> DVE copy ->
        DMA out. Dummy matmuls pinned after each exp keep the PE HAM clock
        gate open through ACT-bound windows.
      * ACT chain order: Square, Ln, Exp(norm), r0 sqrts, r1-g0 sqrts,
        r0 exps, r1-g1/g2 sqrts, r1 exps — 7 table loads, max PE/ACT overlap.
  - Host: scatter-add row/col partial sums, LSE = log(rowsum), npos from
    labels; loss = (masked + sum npos*LSE - N*2/tau) / sum(npos).
"""

import os
import sys

import numpy as np
import ml_dtypes

try:
    import concourse.bass as bass  # noqa: F401
except ImportError:  # harness runs from a bare directory
    for p in ("/opt/trn_rl_repo", os.path.expanduser("~/.axon_site/_ro/trn_rl_repo")):
        if os.path.isdir(p) and p not in sys.path:
            sys.path.insert(0, p)
    import concourse.bass as bass  # noqa: F401

import concourse.mybir as mybir
import concourse.tile as tile
from concourse import bacc, bass_utils
from concourse.tile import add_dep_helper

N, D, NCORES = 8192, 512, 8
RPC = N // NCORES        # 1024 rows per core
NB = RPC // 128          # 8 row blocks of 128
KT = D // 128            # 4 contraction tiles
CH = 512                 # col chunk
NCHL = 10                # local col chunks used (0..9)
SPAN = NCHL * CH         # 5120 local cols
GW = 1536                # sim col group width (3 psum banks)
NCLS = 100               # label classes

# cells: (row-chunk r in {0,1}, local col chunk ch). weight 1 for diagonal
# (ch==r) and antipodal (ch==r+8) cells, else 2 (mirror counted via col-sums).
CELLS = [(0, ch) for ch in range(9)] + [(1, ch) for ch in range(1, 10)]
CELL_W = [1.0 if (ch == r or ch == r + 8) else 2.0 for (r, ch) in CELLS]
CS_CELLS = [(0, ch) for ch in range(1, 8)] + [(1, ch) for ch in range(2, 9)]
# T-matmul chunk sets (3 chunks per [128,1536] psum tile)
T_SETS = {0: [[0, 1, 2], [3, 4, 5], [6, 7, 8]], 1: [[1, 2, 3], [4, 5, 6], [7, 8, 9]]}
# colsum chunk sets (2 chunks per [1,1024] psum tile)
C_SETS = {0: [[1, 2], [3, 4], [5, 6], [7]], 1: [[2, 3], [4, 5], [6, 7], [8]]}

BF16 = mybir.dt.bfloat16
FP8 = mybir.dt.float8e4
FP16 = mybir.dt.float16
FP32 = mybir.dt.float32
OP = mybir.AluOpType
AF = mybir.ActivationFunctionType

_CACHE: dict = {}
last_results = None  # test harness reads exec_time_ns from here


def _build(tau: float):
    nc = bacc.Bacc(
        "TRN2",
        target_bir_lowering=False,
        debug=False,
        enable_asserts=False,
        num_devices=NCORES,
    )
    tok8d = nc.dram_tensor("tok8", [D, SPAN], FP8, kind="ExternalInput")
    ohtd = nc.dram_tensor("oht", [128, SPAN], BF16, kind="ExternalInput")
    ohbd = nc.dram_tensor("ohb", [128, NB * NCLS], FP16, kind="ExternalInput")
    outd = nc.dram_tensor("part", [128, 8 + len(CELLS)], FP32, kind="ExternalOutput")
    colpd = nc.dram_tensor("colp", [1, len(CS_CELLS) * CH], FP32, kind="ExternalOutput")

    A = 2.0 / (tau * tau)  # (dist/tau)^2 = A - A*sim

    act_chain = []  # ACT instructions in required execution order

    def act(*args, **kwargs):
        inst = nc.scalar.activation(*args, **kwargs)
        act_chain.append(inst)
        return inst

    with tile.TileContext(nc) as tc:
        with (
            tc.tile_pool(name="persist", bufs=1) as pp,
            tc.tile_pool(name="dist", bufs=8) as distp,
            tc.tile_pool(name="ep", bufs=4) as ep,
            tc.tile_pool(name="scratch", bufs=2) as sc,
            tc.tile_pool(name="psum", bufs=2, space="PSUM") as psum,
            tc.tile_pool(name="ccb", bufs=2) as ccp,
            tc.tile_pool(name="wpsum", bufs=1, space="PSUM") as wpsum,
            tc.tile_pool(name="init", bufs=1) as initp,
        ):
            # ---- persistent tiles ----
            h8 = [
                pp.tile([128, SPAN], FP8, tag=f"h8_{k}", name=f"h8_{k}")
                for k in range(KT)
            ]
            OHT = pp.tile([128, SPAN], BF16, tag="OHT")
            OHB = pp.tile([128, NB * NCLS], FP16, tag="OHB")
            nrmv = pp.tile([128, SPAN], FP16, tag="nrmv")
            tT8 = [
                pp.tile([128, 2, SPAN], FP8, tag=f"tT8_{t}", name=f"tT8_{t}")
                for t in range(2)
            ]
            dms = pp.tile([128, 4 * 512], BF16, tag="dms")
            idK = pp.tile([128, 128], BF16, tag="idK")
            outp = pp.tile([128, 8 + len(CELLS)], FP32, tag="outp")
            biasA = pp.tile([128, 1], FP32, tag="biasA")
            biasL = pp.tile([128, 1], FP32, tag="biasL")
            ones128 = pp.tile([128, 128], FP16, tag="ones128")
            onesbf = pp.tile([128, 128], BF16, tag="onesbf")

            nc.gpsimd.memset(biasA[:], float(A))
            nc.gpsimd.memset(biasL[:], 2.772588722239781)
            nc.gpsimd.memset(ones128[:], 1.0)
            nc.gpsimd.memset(onesbf[:], 1.0)

            # ---- diag masks dm_k[p, f] = (f - p == 128k) ----
            iot = initp.tile([128, 512], mybir.dt.int32, tag="iot")
            nc.gpsimd.iota(iot[:], pattern=[[1, 512]], base=0, channel_multiplier=-1)
            iotf = initp.tile([128, 512], FP32, tag="iotf")
            nc.vector.tensor_copy(iotf[:], iot[:])
            for kk in range(4):
                nc.vector.tensor_scalar(
                    dms[:, kk * 512:(kk + 1) * 512], iotf[:],
                    float(kk * 128), None, op0=OP.is_equal,
                )
            nc.vector.tensor_scalar_mul(dms[:], dms[:], -512.0)
            nc.vector.tensor_scalar(
                idK[:], iotf[:, 0:128], 0.0, None, op0=OP.is_equal,
            )

            # ---- load tT + norms (GPSIMD square -> PE col sums, broadcast
            # across partitions via ones[128,128] lhsT; inv = exp(-ln(n2)/2)) ----
            for h in range(2):
                hs = slice(h * 2560, (h + 1) * 2560)
                for k in range(KT):
                    nc.sync.dma_start(
                        h8[k][:, hs], tok8d[k * 128:(k + 1) * 128, hs],
                    )
            nc.sync.dma_start(OHT[:], ohtd[:, :])
            nc.sync.dma_start(OHB[:], ohbd[:, :])
            sqs = []
            for k in range(KT):
                sq = distp.tile([128, SPAN], FP16, tag="dist", name=f"sq{k}")
                sqs.append(sq)
            for h in range(2):
                hs = slice(h * 2560, (h + 1) * 2560)
                for k in range(KT):
                    if k % 2 == 0:
                        act(sqs[k][:, hs], h8[k][:, hs], AF.Square)
                    else:
                        nc.vector.tensor_mul(sqs[k][:, hs], h8[k][:, hs],
                                             h8[k][:, hs])
            for gi in range(4):  # 10 chunks in groups of 3 (last has 1)
                chs = [c for c in range(gi * 3, min(gi * 3 + 3, NCHL))]
                w = len(chs) * 512
                ps = psum.tile([128, GW], FP32, tag="ps", name=f"nps{gi}")
                for ci, c in enumerate(chs):
                    for k in range(KT):
                        nc.tensor.matmul(
                            ps[:, ci * 512:(ci + 1) * 512],
                            ones128[:, :],
                            sqs[k][:, c * 512:(c + 1) * 512],
                            start=(k == 0), stop=(k == KT - 1),
                        )
                act(nrmv[:, gi * GW:gi * GW + w], ps[:, 0:w], AF.Ln)
            for h in range(2):  # inv = exp(-0.5*ln(n2)), split for overlap
                hs = slice(h * 2560, (h + 1) * 2560)
                act(nrmv[:, hs], nrmv[:, hs], AF.Exp, scale=-0.5,
                    bias=biasL[:])
            for si, (h0, h1) in enumerate(((0, 1536), (1536, 3456), (3456, 5120))):
                hs = slice(h0, h1)
                for k in range(KT):
                    nc.vector.scalar_tensor_tensor(
                        out=tT8[k // 2][:, k % 2, hs],
                        in0=h8[k][:, hs], scalar=1.0,
                        in1=nrmv[:, hs], op0=OP.mult, op1=OP.mult,
                    )

            # ---- main compute ----
            dist_of = {}
            E_of = {}

            def sim_round(r, groups=(0, 1, 2)):
                off = 512 * r
                if 0 in groups:
                    for m in range(4 * r, 4 * r + 4):
                        dist_of[m] = distp.tile([128, SPAN], FP16, tag="dist",
                                                name=f"dist{m}")
                for g in groups:
                    goff = off + g * GW
                    for m in range(4 * r, 4 * r + 4):
                        dist_m = dist_of[m]
                        ps = psum.tile([128, GW], FP32, tag="ps",
                                       name=f"ps{m}_{g}")
                        for t in range(2):
                            lhsT = tT8[t][:, :, m * 128:(m + 1) * 128]
                            for s in range(GW // 512):
                                nc.tensor.matmul(
                                    ps[:, s * 512:(s + 1) * 512],
                                    lhsT,
                                    tT8[t][:, :, goff + s * 512: goff + (s + 1) * 512],
                                    start=(t == 0),
                                    stop=(t == 1) and not (g == 0 and s == 0),
                                    perf_mode=mybir.MatmulPerfMode.DoubleRow,
                                )
                        if g == 0:
                            # diag fix via PE: accumulate idK.T @ (-512*mask)
                            nc.tensor.matmul(
                                ps[:, 0:512], idK[:, :],
                                dms[:, (m % 4) * 512:(m % 4 + 1) * 512],
                                start=False, stop=True,
                            )
                        act(dist_m[:, goff:goff + GW], ps[:, :], AF.Sqrt,
                            bias=biasA[:], scale=float(-A / 256.0))

            def t_round(r):
                blocks = list(range(4 * r, 4 * r + 4))
                for si, tset in enumerate(T_SETS[r]):
                    tps = psum.tile([128, GW], FP32, tag="ps",
                                    name=f"tps{r}_{si}")
                    for mi, m in enumerate(blocks):
                        for ci, c in enumerate(tset):
                            nc.tensor.matmul(
                                tps[0:NCLS, ci * 512:(ci + 1) * 512],
                                OHB[:, m * NCLS:(m + 1) * NCLS],
                                dist_of[m][:, c * 512:(c + 1) * 512],
                                start=(mi == 0), stop=(mi == 3),
                            )
                    for ci, c in enumerate(tset):
                        cell = CELLS.index((r, c))
                        tjunk = sc.tile([128, 512], FP16, tag="tjunk")
                        nc.vector.scalar_tensor_tensor(
                            out=tjunk[0:NCLS, :],
                            in0=tps[0:NCLS, ci * 512:(ci + 1) * 512],
                            scalar=1.0,
                            in1=OHT[0:NCLS, c * 512:(c + 1) * 512],
                            op0=OP.mult, op1=OP.mult,
                            accum_out=outp[0:NCLS, 8 + cell:9 + cell],
                        )

            def exp_block(m):
                off = 512 * (m // 4)
                E_m = ep.tile([128, SPAN], BF16, tag="E", name=f"E{m}")
                E_of[m] = E_m
                e = act(E_m[:, off:off + 9 * 512],
                        dist_of[m][:, off:off + 9 * 512], AF.Exp, scale=-1.0,
                        accum_out=outp[:, m:m + 1])
                # HAM keep-warm: tiny matmul pinned after this exp so the PE
                # clock gate stays open through the ACT-bound window
                wps = wpsum.tile([128, 512], FP32, tag="wps", name=f"wps{m}")
                w = nc.tensor.matmul(wps[:, :], idK[:, :], dms[:, 0:512],
                                     start=True, stop=True)
                add_dep_helper(w.ins, e.ins, reason="ham keep-warm")

            def exp_round(r):
                for m in range(4 * r, 4 * r + 4):
                    exp_block(m)

            def colsum_round(r):
                blocks = list(range(4 * r, 4 * r + 4))
                chunks = [c for cset in C_SETS[r] for c in cset]  # 7 contiguous
                for si in range(3):  # 3+3+1 chunk slices per psum tile
                    cset = chunks[si * 3:si * 3 + 3]
                    if not cset:
                        continue
                    w = len(cset) * 512
                    cps = psum.tile([128, GW], FP32, tag="ps",
                                    name=f"cps{r}_{si}")
                    for ci, c in enumerate(cset):
                        for mi, m in enumerate(blocks):
                            nc.tensor.matmul(
                                cps[:, ci * 512:(ci + 1) * 512],
                                onesbf[:, :],
                                E_of[m][:, c * 512:(c + 1) * 512],
                                start=(mi == 0), stop=(mi == 3),
                            )
                    cc = ccp.tile([1, GW], FP32, tag="cc")
                    nc.vector.tensor_copy(cc[0:1, 0:w], cps[0:1, 0:w])
                    slot = CS_CELLS.index((r, cset[0]))
                    nc.sync.dma_start(
                        colpd[0:1, slot * 512:slot * 512 + w], cc[0:1, 0:w],
                    )

            sim_round(0)
            t_round(0)
            sim_round(1, groups=(0,))
            exp_round(0)
            sim_round(1, groups=(1, 2))
            colsum_round(0)
            t_round(1)
            exp_round(1)
            colsum_round(1)

            nc.sync.dma_start(outd[:, :], outp[:])

            # ---- pin ACT execution order (stop table-set thrash) ----
            for a, b in zip(act_chain, act_chain[1:]):
                add_dep_helper(b.ins, a.ins, reason="act table-set order")

    nc.compile()
    return nc


def _get_program(tau: float):
    if tau not in _CACHE:
        _CACHE[tau] = _build(tau)
    return _CACHE[tau]


def make_in_maps(tokens: np.ndarray, labels: np.ndarray):
    bf = ml_dtypes.bfloat16
    f8 = ml_dtypes.float8_e4m3fn
    tokT_full = (np.ascontiguousarray(
        np.asarray(tokens, dtype=np.float32).T
    ) * 16.0).astype(f8)                           # [D, N] fp8, x16
    lab = np.asarray(labels).astype(np.int64)
    vids = np.arange(128)
    in_maps = []
    for c in range(NCORES):
        sh = c * RPC
        t8 = np.ascontiguousarray(np.roll(tokT_full, -sh, axis=1)[:, :SPAN])
        lab_loc = np.roll(lab, -sh)[:SPAN]
        oht = (vids[:, None] == lab_loc[None, :]).astype(bf)
        lab_blk = lab_loc[:RPC].reshape(NB, 128)
        ohb = np.zeros((128, NB * NCLS), dtype=np.float16)
        for m in range(NB):
            ohb[:, m * NCLS:(m + 1) * NCLS] = (
                lab_blk[m][:, None] == np.arange(NCLS)[None, :]
            )
        in_maps.append({"tok8": t8, "oht": oht, "ohb": ohb})
    return in_maps


def _install_ntff_hook_shim():
    """Provide antenv.axon_hooks if the image lacks it (NTFF profiling via
    direct ctypes calls into libaxon_pjrt.so)."""
    try:
        from antenv.axon_hooks import get_axon_ntff_profile_hook  # noqa: F401
        return True
    except ImportError:
        pass
    so_path = "/opt/axon/libaxon_pjrt.so"
    if not os.path.exists(so_path):
        return False
    import contextlib
    import ctypes
    import types

    lib = ctypes.CDLL(so_path)
    if not hasattr(lib, "axon_start_nrt_profile"):
        return False
    lib.axon_start_nrt_profile.argtypes = [
        ctypes.POINTER(ctypes.c_int64), ctypes.c_size_t,
    ]
    lib.axon_start_nrt_profile.restype = ctypes.c_int64
    lib.axon_stop_nrt_profile.argtypes = [ctypes.c_char_p]
    lib.axon_stop_nrt_profile.restype = ctypes.c_int64

    @contextlib.contextmanager
    def _hook(output_dir, device_ids):
        import jax
        jax.devices()
        if device_ids:
            ids = (ctypes.c_int64 * len(device_ids))(*device_ids)
            rc = lib.axon_start_nrt_profile(ids, len(device_ids))
        else:
            rc = lib.axon_start_nrt_profile(None, 0)
        if rc != 0:
            raise RuntimeError(f"axon_start_nrt_profile rc={rc}")
        try:
            yield
        finally:
            n = lib.axon_stop_nrt_profile(str(output_dir).encode())
            if n < 0:
                raise RuntimeError(f"axon_stop_nrt_profile rc={n}")
            print(f"profile: {n} file(s) written to {output_dir}")

    mod = types.ModuleType("antenv.axon_hooks")
    mod.get_axon_ntff_profile_hook = lambda: _hook
    mod.set_axon_ntff_profile_hook = lambda h: None
    sys.modules["antenv.axon_hooks"] = mod
    return True


def kernel(tokens, labels, temperature=0.07):
    global last_results
    tau = float(temperature)
    nc = _get_program(tau)
    lab = np.asarray(labels).astype(np.int64)
    in_maps = make_in_maps(tokens, lab)
    trace = bool(int(os.environ.get("KBENCH_TRACE", "0")))
    if trace:
        trace = _install_ntff_hook_shim()
    res = bass_utils.run_bass_kernel_spmd(
        nc, in_maps, core_ids=list(range(NCORES)),
        trace=trace,
    )
    last_results = res

    rs_g = np.zeros(N, dtype=np.float64)
    num = 0.0
    for c in range(NCORES):
        sh = c * RPC
        part = res.results[c]["part"].astype(np.float64)
        colp = res.results[c]["colp"].astype(np.float64)
        rs_g[sh:sh + RPC] += part[:, 0:8].T.reshape(RPC)
        for slot, (r, ch) in enumerate(CS_CELLS):
            gcols = (sh + ch * 512 + np.arange(512)) % N
            rs_g[gcols] += colp[0, slot * 512:(slot + 1) * 512]
        for ci, (r, ch) in enumerate(CELLS):
            num += CELL_W[ci] * part[0:NCLS, 8 + ci].sum()
    cnt = np.bincount(lab, minlength=NCLS)
    npos = (cnt[lab] - 1).astype(np.float64)
    num += (npos * np.log(rs_g)).sum() - N * (2.0 / tau)
    den = npos.sum()
    return np.float32(num / den)
